# revision 13
# baseline (speedup 1.0000x reference)
"""DN4 metric kernel for Trainium2 (8 NeuronCores, SPMD via bass/Tile).

Computes, for fm [100, 64, 21, 21] (bs=2, nw=5, ns=5, nq=5):
  fm_hat = fm / (||fm||_c + 1e-12)   (L2 over channel axis per spatial pos)
  sim[b,q,w,x,y] = <que_hat[b,q,x,:], sup_hat[b,w,y,:]>
  pred[b,q,w] = sum_x sum(top3_y sim)
  el = elabel.reshape(bs,nw,ns+nq)[:,:,ns:].reshape(-1)

Device algorithm (per core, 36 slots of one (b,w,q) combo each):
  - supports normalized on device once per bank (4 banks of [64, 2205])
  - queries left unnormalized; sim_raw = que^T @ sup_hat (float32r matmuls)
  - per 112-row x-chunk: ScalarE evicts PSUM->SBUF, DVE `max` gives top-8 per
    row in one pass, reduce first 3 -> top3sums
  - 1/||q_x|| via PE transpose + square + reduce + sqrt + reciprocal; final
    pred[slot] = sum_x invq[x]*top3[x] computed as a tiny matmul (partition
    reduction) accumulated in PSUM.
"""

import numpy as np

import concourse.bacc as bacc
import concourse.bass as bass
import concourse.mybir as mybir
from concourse.masks import make_identity
from concourse.tile import TileContext
from concourse.bass_utils import run_bass_kernel_spmd

# problem constants (hardcoded per spec)
BS, NW, NS, NQ = 2, 5, 5, 5
C, HW = 64, 441
Y = NS * HW            # 2205 support descriptors per way
QP = 448               # que spatial padded to 4*112
XP = 112               # x-chunk partition size
XCH = 4                # x chunks per combo
NCORES = 8
BANKS = 4              # sup groups resident per core
SPB = 9                # slots per bank
SLOTS = BANKS * SPB    # 36
N_GROUPS = BS * NW     # 10
GROUP_SPLITS = (9, 8, 8)   # how each group's 25 q's split into bank-chunks
USE_F32R = True

F32 = mybir.dt.float32
F32R = mybir.dt.float32r
AF = mybir.ActivationFunctionType

_CACHE = {}


def _sim_chunks():
    # (offset, width) N-chunks of the 2205-wide sim row; <=512 each (PSUM bank)
    return [(0, 512), (512, 512), (1024, 512), (1536, 512), (2048, 157)]


# matmul N-chunks for the sim rows: all 512 wide (fp32r wants wide, even N;
# 512 also hits the 1 cycle/row fp32r fast path). The last chunk overlaps the
# fourth by 355 columns — identical values, and only its new 157 columns are
# evicted, so no duplicates reach the top-k.
MM_CHUNKS = [(0, 512, 0, 512), (512, 512, 0, 512), (1024, 512, 0, 512),
             (1536, 512, 0, 512), (1693, 512, 355, 157)]


def _build_program():
    nc = bacc.Bacc("TRN2", target_bir_lowering=False, debug=False,
                   num_devices=NCORES)
    que_in = nc.declare_dram_parameter("que", [SLOTS, C, QP],
                                       F32R if USE_F32R else F32, isOutput=False)
    sup_in = nc.declare_dram_parameter("supraw", [BANKS, C, Y], F32, isOutput=False)
    out = nc.declare_dram_parameter("pred_part", [1, SLOTS], F32, isOutput=True)

    mmdt = F32R if USE_F32R else F32

    with TileContext(nc) as tc:
        with tc.tile_pool(name="persist", bufs=1) as persist, \
             tc.tile_pool(name="ppsum", bufs=1, space="PSUM") as ppsum, \
             tc.tile_pool(name="pre", bufs=2) as pre, \
             tc.tile_pool(name="mq", bufs=3) as mq, \
             tc.tile_pool(name="msim", bufs=3) as msim, \
             tc.tile_pool(name="msmall", bufs=4) as msmall, \
             tc.tile_pool(name="mpsA", bufs=2, space="PSUM") as mpsA, \
             tc.tile_pool(name="mpsB", bufs=2, space="PSUM") as mpsB, \
             tc.tile_pool(name="mpsQ", bufs=1, space="PSUM") as mpsQ:
            identity = persist.tile([64, 64], F32, tag="ident")
            make_identity(nc, identity)
            ones64 = persist.tile([C, 1], F32, tag="ones")
            nc.vector.memset(ones64, 1.0)
            shat_banks = [persist.tile([C, Y], mmdt, tag=f"shat{b}",
                                       name=f"shat{b}")
                          for b in range(BANKS)]
            pred_ps = ppsum.tile([1, 64], F32, tag="pred")
            top8_all = persist.tile([XP, SLOTS, XCH, 8], F32, tag="top8a")
            invq_all = persist.tile([XP, SLOTS, XCH], F32, tag="invqa")

            def emit_bank_norm(b):
                # normalize support bank b: shat[c, y] = sup[c, y] / ||s_y||
                raw = pre.tile([C, Y], F32, tag="raw")
                nc.sync.dma_start(out=raw, in_=sup_in[b])
                sq = pre.tile([C, Y], F32, tag="sq")
                nc.scalar.activation(sq, raw, AF.Square)
                ssp = mpsB.tile([XP, 512], F32, tag="pB")   # borrow a bank
                ss = pre.tile([1, Y], F32, tag="ss")
                for ci, (o, w) in enumerate(_sim_chunks()):
                    nc.tensor.matmul(ssp[0:1, 0:w], lhsT=ones64,
                                     rhs=sq[:, o:o + w], start=True, stop=True)
                    nc.scalar.activation(ss[0:1, o:o + w], ssp[0:1, 0:w], AF.Copy)
                ssr = pre.tile([15, 147], F32, tag="ssr")   # 15*147 == 2205
                nc.sync.dma_start(out=ssr, in_=ss[0:1, :])
                ssq = pre.tile([15, 147], F32, tag="ssq")
                nc.scalar.activation(ssq, ssr, AF.Sqrt)     # ||s_y||
                rr = pre.tile([15, 147], F32, tag="rr")
                nc.vector.reciprocal(rr, ssq)               # 1/||s_y||
                rflat = pre.tile([1, Y], F32, tag="rflat")
                nc.sync.dma_start(out=rflat[0:1, :], in_=rr)
                rb = pre.tile([C, Y], F32, tag="rb")
                nc.sync.dma_start(out=rb[0:1, :], in_=rflat[0:1, :])
                p = 1
                while p < C:
                    step = min(p, C - p)
                    nc.sync.dma_start(out=rb[p:p + step, :], in_=rb[0:step, :])
                    p += step
                nc.gpsimd.tensor_mul(shat_banks[b], raw, rb)

            def emit_slot(s):
                bank = s // SPB
                que_t = mq.tile([C, QP], mmdt, tag="que")
                nc.sync.dma_start(out=que_t, in_=que_in[s])

                # query norms: transpose 4 x-chunks -> [112, 4, 64] PSUM
                qT = mpsQ.tile([XP, XCH, 64], F32, tag="qT")
                for j in range(XCH):
                    nc.tensor.transpose(qT[:, j, :],
                                        que_t[:, j * XP:(j + 1) * XP].bitcast(F32),
                                        identity)
                sqs = msmall.tile([XP, XCH, 64], F32, tag="sqs")
                nc.scalar.activation(sqs, qT, AF.Square)
                ss4 = msmall.tile([XP, XCH], F32, tag="ss4")
                nc.vector.reduce_sum(ss4, sqs, axis=mybir.AxisListType.X)
                nrm = msmall.tile([XP, XCH], F32, tag="nrm")
                nc.scalar.activation(nrm, ss4, AF.Sqrt)
                nc.vector.reciprocal(invq_all[:, s, :], nrm)

                for j in range(XCH):
                    lhs = que_t[:, j * XP:(j + 1) * XP]
                    pA1 = mpsA.tile([XP, 1024], F32, tag="pA")
                    pA2 = mpsA.tile([XP, 1024], F32, tag="pA")
                    pB = mpsB.tile([XP, 512], F32, tag="pB")
                    for ci, (o, w, _eo, _ew) in enumerate(MM_CHUNKS):
                        dst = (pA1[:, 0:512], pA1[:, 512:1024],
                               pA2[:, 0:512], pA2[:, 512:1024],
                               pB[:, 0:512])[ci]
                        nc.tensor.matmul(
                            dst, lhsT=lhs,
                            rhs=shat_banks[bank][:, o:o + w],
                            start=True, stop=True)
                    sim_sb = msim.tile([XP, Y], F32, tag="sim")
                    nc.scalar.activation(sim_sb[:, 0:1024], pA1, AF.Copy)
                    nc.scalar.activation(sim_sb[:, 1024:2048], pA2, AF.Copy)
                    nc.scalar.activation(sim_sb[:, 2048:2205], pB[:, 355:512],
                                         AF.Copy)
                    nc.vector.max(out=top8_all[:, s, j, :], in_=sim_sb)

            for b in range(BANKS):
                emit_bank_norm(b)
                for s in range(b * SPB, (b + 1) * SPB):
                    emit_slot(s)

            # batched top-3 sums + per-slot weighted partition reduction
            top3_all = msmall.tile([XP, SLOTS * XCH], F32, tag="top3a")
            nc.vector.reduce_sum(top3_all, top8_all[:, :, :, 0:3],
                                 axis=mybir.AxisListType.X)
            for s in range(SLOTS):
                for j in range(XCH):
                    # last x-chunk only has 105 real rows (441 = 3*112+105);
                    # restricting K keeps inf/junk pad lanes out of the sum
                    kp = 105 if j == XCH - 1 else XP
                    col = s * XCH + j
                    nc.tensor.matmul(pred_ps[0:1, s:s + 1],
                                     lhsT=invq_all[0:kp, s, j:j + 1],
                                     rhs=top3_all[0:kp, col:col + 1],
                                     start=(j == 0), stop=(j == XCH - 1))

            predsb = msmall.tile([1, SLOTS], F32, tag="predsb")
            nc.scalar.activation(predsb, pred_ps[0:1, 0:SLOTS], AF.Copy)
            nc.sync.dma_start(out=out[:], in_=predsb)

    nc.compile()
    return nc


def _slot_table():
    """Returns per-core lists: banks[core] = [group]*4, slots[core] =
    [(bank, group, q) or None] * SLOTS."""
    chunks = []  # (group, qlo, qhi)
    for g in range(N_GROUPS):
        q0 = 0
        for w in GROUP_SPLITS:
            chunks.append((g, q0, q0 + w))
            q0 += w
    banks = [[0] * BANKS for _ in range(NCORES)]
    slots = [[None] * SLOTS for _ in range(NCORES)]
    for k in range(NCORES):
        for b in range(BANKS):
            idx = k * BANKS + b
            if idx < len(chunks):
                g, qlo, qhi = chunks[idx]
            else:
                g, qlo, qhi = 0, 0, 0   # idle bank
            banks[k][b] = g
            for j in range(SPB):
                q = qlo + j
                slots[k][b * SPB + j] = (g, q) if q < qhi else None
    return banks, slots


def kernel(fm, elabel, glabel, bs, nw, ns, nq):
    fm = np.ascontiguousarray(np.asarray(fm, dtype=np.float32))
    elabel = np.asarray(elabel)
    assert fm.shape == (BS * NW * (NS + NQ), C, 21, 21), fm.shape
    assert int(bs) == BS and int(nw) == NW and int(ns) == NS and int(nq) == NQ

    fmf = fm.reshape(BS * NW * (NS + NQ), C, HW)
    banks, slots = _slot_table()

    in_maps = []
    for k in range(NCORES):
        sup = np.empty((BANKS, C, Y), np.float32)
        for b in range(BANKS):
            g = banks[k][b]
            b_idx, w = g // NW, g % NW
            imgs = [fmf[(b_idx * NW + w) * (NS + NQ) + s] for s in range(NS)]
            sup[b] = np.concatenate(imgs, axis=1)
        que = np.zeros((SLOTS, C, QP), np.float32)
        for s in range(SLOTS):
            if slots[k][s] is None:
                continue
            g, q = slots[k][s]
            b_idx = g // NW
            n = (b_idx * NW + q // NQ) * (NS + NQ) + NS + q % NQ
            que[s, :, :HW] = fmf[n]
        in_maps.append({"que": que, "supraw": sup})

    if "nc" not in _CACHE:
        _CACHE["nc"] = _build_program()
    nc = _CACHE["nc"]

    res = run_bass_kernel_spmd(nc, in_maps, list(range(NCORES)))
    _CACHE["last_results"] = res

    pred = np.zeros((BS, NW * NQ, NW), np.float32)
    for k in range(NCORES):
        part = res.results[k]["pred_part"][0]
        for s in range(SLOTS):
            if slots[k][s] is None:
                continue
            g, q = slots[k][s]
            b_idx, w = g // NW, g % NW
            pred[b_idx, q, w] = part[s]

    el = elabel.reshape(BS, NW, NS + NQ)[:, :, NS:].reshape(-1)
    return pred, el


# revision 15
# speedup vs baseline: 1.2083x; 1.2083x over previous
"""DN4 metric kernel for Trainium2 (8 NeuronCores, SPMD via bass/Tile).

Computes, for fm [100, 64, 21, 21] (bs=2, nw=5, ns=5, nq=5):
  fm_hat = fm / (||fm||_c + 1e-12)   (L2 over channel axis per spatial pos)
  sim[b,q,w,x,y] = <que_hat[b,q,x,:], sup_hat[b,w,y,:]>
  pred[b,q,w] = sum_x sum(top3_y sim)
  el = elabel.reshape(bs,nw,ns+nq)[:,:,ns:].reshape(-1)

Device algorithm (per core, 36 slots of one (b,w,q) combo each):
  - supports normalized on device once per bank (4 banks of [64, 2205])
  - queries left unnormalized; sim_raw = que^T @ sup_hat (float32r matmuls)
  - per 112-row x-chunk: ScalarE evicts PSUM->SBUF, DVE `max` gives top-8 per
    row in one pass, reduce first 3 -> top3sums
  - 1/||q_x|| via PE transpose + square + reduce + sqrt + reciprocal; final
    pred[slot] = sum_x invq[x]*top3[x] computed as a tiny matmul (partition
    reduction) accumulated in PSUM.
"""

import numpy as np

import concourse.bacc as bacc
import concourse.bass as bass
import concourse.mybir as mybir
from concourse.masks import make_identity
from concourse.tile import TileContext
from concourse.bass_utils import run_bass_kernel_spmd

# problem constants (hardcoded per spec)
BS, NW, NS, NQ = 2, 5, 5, 5
C, HW = 64, 441
Y = NS * HW            # 2205 support descriptors per way
QP = 448               # que spatial padded to 4*112
XP = 112               # x-chunk partition size
XCH = 4                # x chunks per combo
NCORES = 8
BANKS = 4              # sup groups resident per core
SPB = 9                # slots per bank
SLOTS = BANKS * SPB    # 36
N_GROUPS = BS * NW     # 10
GROUP_SPLITS = (9, 8, 8)   # how each group's 25 q's split into bank-chunks
USE_F32R = True

F32 = mybir.dt.float32
F32R = mybir.dt.float32r
AF = mybir.ActivationFunctionType

_CACHE = {}


def _sim_chunks():
    # (offset, width) N-chunks of the 2205-wide sim row; <=512 each (PSUM bank)
    return [(0, 512), (512, 512), (1024, 512), (1536, 512), (2048, 157)]


# matmul N-chunks for the sim rows: all 512 wide (fp32r wants wide, even N;
# 512 also hits the 1 cycle/row fp32r fast path). The last chunk overlaps the
# fourth by 355 columns — identical values, and only its new 157 columns are
# evicted, so no duplicates reach the top-k.
MM_CHUNKS = [(0, 512, 0, 512), (512, 512, 0, 512), (1024, 512, 0, 512),
             (1536, 512, 0, 512), (1693, 512, 355, 157)]


def _build_program():
    nc = bacc.Bacc("TRN2", target_bir_lowering=False, debug=False,
                   num_devices=NCORES)
    que_in = nc.declare_dram_parameter("que", [SLOTS, C, QP],
                                       F32R if USE_F32R else F32, isOutput=False)
    sup_in = nc.declare_dram_parameter("supraw", [BANKS, C, Y], F32, isOutput=False)
    out = nc.declare_dram_parameter("pred_part", [1, SLOTS], F32, isOutput=True)

    mmdt = F32R if USE_F32R else F32

    with TileContext(nc) as tc:
        with tc.tile_pool(name="persist", bufs=1) as persist, \
             tc.tile_pool(name="ppsum", bufs=1, space="PSUM") as ppsum, \
             tc.tile_pool(name="pre", bufs=2) as pre, \
             tc.tile_pool(name="mq", bufs=3) as mq, \
             tc.tile_pool(name="msim", bufs=2) as msim, \
             tc.tile_pool(name="msmall", bufs=4) as msmall, \
             tc.tile_pool(name="mpsA", bufs=2, space="PSUM") as mpsA, \
             tc.tile_pool(name="mpsB", bufs=1, space="PSUM") as mpsB, \
             tc.tile_pool(name="mpsQ", bufs=1, space="PSUM") as mpsQ, \
             tc.tile_pool(name="prep", bufs=1, space="PSUM") as prep:
            identity = persist.tile([64, 64], F32, tag="ident")
            make_identity(nc, identity)
            ones64 = persist.tile([C, 1], F32, tag="ones")
            nc.vector.memset(ones64, 1.0)
            shat_banks = [persist.tile([C, Y], mmdt, tag=f"shat{b}",
                                       name=f"shat{b}")
                          for b in range(BANKS)]
            pred_ps = ppsum.tile([1, 64], F32, tag="pred")
            top8_all = persist.tile([XP, SLOTS, XCH, 8], F32, tag="top8a")
            invq_all = persist.tile([XP, SLOTS, XCH], F32, tag="invqa")

            def emit_bank_norm(b):
                # normalize support bank b: shat[c, y] = sup[c, y] / ||s_y||
                raw = pre.tile([C, Y], F32, tag="raw")
                nc.sync.dma_start(out=raw, in_=sup_in[b])
                sq = pre.tile([C, Y], F32, tag="sq")
                nc.scalar.activation(sq, raw, AF.Square)
                ssp = prep.tile([1, 512], F32, tag="ssp")
                ss = pre.tile([1, Y], F32, tag="ss")
                for ci, (o, w) in enumerate(_sim_chunks()):
                    nc.tensor.matmul(ssp[0:1, 0:w], lhsT=ones64,
                                     rhs=sq[:, o:o + w], start=True, stop=True)
                    nc.scalar.activation(ss[0:1, o:o + w], ssp[0:1, 0:w], AF.Copy)
                ssr = pre.tile([15, 147], F32, tag="ssr")   # 15*147 == 2205
                nc.sync.dma_start(out=ssr, in_=ss[0:1, :])
                ssq = pre.tile([15, 147], F32, tag="ssq")
                nc.scalar.activation(ssq, ssr, AF.Sqrt)     # ||s_y||
                rr = pre.tile([15, 147], F32, tag="rr")
                nc.vector.reciprocal(rr, ssq)               # 1/||s_y||
                rflat = pre.tile([1, Y], F32, tag="rflat")
                nc.sync.dma_start(out=rflat[0:1, :], in_=rr)
                rb = pre.tile([C, Y], F32, tag="rb")
                nc.sync.dma_start(out=rb[0:1, :], in_=rflat[0:1, :])
                p = 1
                while p < C:
                    step = min(p, C - p)
                    nc.sync.dma_start(out=rb[p:p + step, :], in_=rb[0:step, :])
                    p += step
                nc.gpsimd.tensor_mul(shat_banks[b], raw, rb)

            def emit_slot(s):
                bank = s // SPB
                que_t = mq.tile([C, QP], mmdt, tag="que")
                nc.sync.dma_start(out=que_t, in_=que_in[s])

                # query norms: transpose 4 x-chunks -> [112, 4, 64] PSUM
                qT = mpsQ.tile([XP, XCH, 64], F32, tag="qT")
                for j in range(XCH):
                    nc.tensor.transpose(qT[:, j, :],
                                        que_t[:, j * XP:(j + 1) * XP].bitcast(F32),
                                        identity)
                sqs = msmall.tile([XP, XCH, 64], F32, tag="sqs")
                nc.scalar.activation(sqs, qT, AF.Square)
                ss4 = msmall.tile([XP, XCH], F32, tag="ss4")
                nc.vector.reduce_sum(ss4, sqs, axis=mybir.AxisListType.X)
                nrm = msmall.tile([XP, XCH], F32, tag="nrm")
                nc.scalar.activation(nrm, ss4, AF.Sqrt)
                nc.vector.reciprocal(invq_all[:, s, :], nrm)

                for j in range(XCH):
                    lhs = que_t[:, j * XP:(j + 1) * XP]
                    pA1 = mpsA.tile([XP, 1024], F32, tag="pA")
                    pA2 = mpsA.tile([XP, 1024], F32, tag="pA")
                    pB = mpsB.tile([XP, 512], F32, tag="pB")
                    for ci, (o, w, _eo, _ew) in enumerate(MM_CHUNKS):
                        dst = (pA1[:, 0:512], pA1[:, 512:1024],
                               pA2[:, 0:512], pA2[:, 512:1024],
                               pB[:, 0:512])[ci]
                        nc.tensor.matmul(
                            dst, lhsT=lhs,
                            rhs=shat_banks[bank][:, o:o + w],
                            start=True, stop=True)
                    sim_sb = msim.tile([XP, Y], F32, tag="sim")
                    nc.scalar.activation(sim_sb[:, 0:1024], pA1, AF.Copy)
                    nc.scalar.activation(sim_sb[:, 1024:2048], pA2, AF.Copy)
                    nc.scalar.activation(sim_sb[:, 2048:2205], pB[:, 355:512],
                                         AF.Copy)
                    nc.vector.max(out=top8_all[:, s, j, :], in_=sim_sb)

            for b in range(BANKS):
                emit_bank_norm(b)
            for s in range(SLOTS):
                emit_slot(s)

            # batched top-3 sums + per-slot weighted partition reduction
            top3_all = msmall.tile([XP, SLOTS * XCH], F32, tag="top3a")
            nc.vector.reduce_sum(top3_all, top8_all[:, :, :, 0:3],
                                 axis=mybir.AxisListType.X)
            for s in range(SLOTS):
                for j in range(XCH):
                    # last x-chunk only has 105 real rows (441 = 3*112+105);
                    # restricting K keeps inf/junk pad lanes out of the sum
                    kp = 105 if j == XCH - 1 else XP
                    col = s * XCH + j
                    nc.tensor.matmul(pred_ps[0:1, s:s + 1],
                                     lhsT=invq_all[0:kp, s, j:j + 1],
                                     rhs=top3_all[0:kp, col:col + 1],
                                     start=(j == 0), stop=(j == XCH - 1))

            predsb = msmall.tile([1, SLOTS], F32, tag="predsb")
            nc.scalar.activation(predsb, pred_ps[0:1, 0:SLOTS], AF.Copy)
            nc.sync.dma_start(out=out[:], in_=predsb)

    nc.compile()
    return nc


def _slot_table():
    """Returns per-core lists: banks[core] = [group]*4, slots[core] =
    [(bank, group, q) or None] * SLOTS."""
    chunks = []  # (group, qlo, qhi)
    for g in range(N_GROUPS):
        q0 = 0
        for w in GROUP_SPLITS:
            chunks.append((g, q0, q0 + w))
            q0 += w
    banks = [[0] * BANKS for _ in range(NCORES)]
    slots = [[None] * SLOTS for _ in range(NCORES)]
    for k in range(NCORES):
        for b in range(BANKS):
            idx = k * BANKS + b
            if idx < len(chunks):
                g, qlo, qhi = chunks[idx]
            else:
                g, qlo, qhi = 0, 0, 0   # idle bank
            banks[k][b] = g
            for j in range(SPB):
                q = qlo + j
                slots[k][b * SPB + j] = (g, q) if q < qhi else None
    return banks, slots


def kernel(fm, elabel, glabel, bs, nw, ns, nq):
    fm = np.ascontiguousarray(np.asarray(fm, dtype=np.float32))
    elabel = np.asarray(elabel)
    assert fm.shape == (BS * NW * (NS + NQ), C, 21, 21), fm.shape
    assert int(bs) == BS and int(nw) == NW and int(ns) == NS and int(nq) == NQ

    fmf = fm.reshape(BS * NW * (NS + NQ), C, HW)
    banks, slots = _slot_table()

    in_maps = []
    for k in range(NCORES):
        sup = np.empty((BANKS, C, Y), np.float32)
        for b in range(BANKS):
            g = banks[k][b]
            b_idx, w = g // NW, g % NW
            imgs = [fmf[(b_idx * NW + w) * (NS + NQ) + s] for s in range(NS)]
            sup[b] = np.concatenate(imgs, axis=1)
        que = np.zeros((SLOTS, C, QP), np.float32)
        for s in range(SLOTS):
            if slots[k][s] is None:
                continue
            g, q = slots[k][s]
            b_idx = g // NW
            n = (b_idx * NW + q // NQ) * (NS + NQ) + NS + q % NQ
            que[s, :, :HW] = fmf[n]
        in_maps.append({"que": que, "supraw": sup})

    if "nc" not in _CACHE:
        _CACHE["nc"] = _build_program()
    nc = _CACHE["nc"]

    res = run_bass_kernel_spmd(nc, in_maps, list(range(NCORES)))
    _CACHE["last_results"] = res

    pred = np.zeros((BS, NW * NQ, NW), np.float32)
    for k in range(NCORES):
        part = res.results[k]["pred_part"][0]
        for s in range(SLOTS):
            if slots[k][s] is None:
                continue
            g, q = slots[k][s]
            b_idx, w = g // NW, g % NW
            pred[b_idx, q, w] = part[s]

    el = elabel.reshape(BS, NW, NS + NQ)[:, :, NS:].reshape(-1)
    return pred, el
